# revision 19
# baseline (speedup 1.0000x reference)
"""Multi-head attention (B=4, S=2048, D=1024, H=16, dk=dv=64) on 8 TRN2 NeuronCores.

Sharding: batch x head-half. Core c handles batch b = c//2 and heads
hh*8..hh*8+8 where hh = c%2. Each core computes its 8 heads' attention plus
the partial output projection (row-parallel fc); the host sums the two
partials per batch and adds the output bias.

Device algorithm per core (matmul inputs bf16, PSUM fp32):
  - host pre-transposes inputs (xT) so every matmul contracts over partitions.
  - Q^T/K^T laid out [feat, seq] with HEAD PAIRS packed on the partition axis
    (even head of a pair on partitions 0..63, odd head on 64..127).
  - scores S^T = K^T' Q^T per head run as K=64 matmuls in PE ROW TILES
    (tile_position (0,0) / (64,0)): the two heads of a pair execute
    CONCURRENTLY in the two array halves, halving score matmul time.
  - scores are emitted in kc-groups of 4 per pair, alternating with the
    128-mode PV block for that group, to amortize PE tiling-mode switches.
  - V augmented with a ones column per head: the softmax denominator rides
    the PV matmul; exp on ScalarE reads score PSUM directly (no max-sub).
  - normalize via reciprocal + partition broadcast (DRAM bounce, off the
    critical path); out_partial = C^T.T @ Wo_c^T, fp32 to DRAM.

PSUM budget (8 banks): score/wedge ring 2x[128,1024] (4 banks; the
projection and output-projection wedges BORROW this ring since the PE can
only run one thing at a time) + PV accumulators 4x[128,512] (4 banks,
two heads of a pair in flight).
"""

import sys

if "/opt/trn_rl_repo" not in sys.path:
    sys.path.insert(0, "/opt/trn_rl_repo")

from contextlib import ExitStack

import ml_dtypes
import numpy as np

import concourse.bass as bass
import concourse.tile as tile
from concourse import bacc, mybir
from concourse.bass_utils import run_bass_kernel_spmd

BF16 = mybir.dt.bfloat16
F32 = mybir.dt.float32
P = 128

B, S, D = 4, 2048, 1024
H, DH = 16, 64
G = 512          # head-group width per core: 8 heads x 64
NH = G // DH     # 8 heads per core
NP = NH // 2     # head pairs per core
DC = D // P      # contraction chunks over model dim
FC = G // P      # feat chunks of the head-group width
SCALE = 1.0 / 8.0  # 1/sqrt(dk)
GK = 2           # kc chunks per scores/PV phase block (mode-switch batching)


def _emit(ctx, tc, io, seq):
    nc = tc.nc
    KC = seq // P                 # key chunks
    QW = min(1024, seq)           # q width per score-psum tile (2 PSUM banks)
    NI = QW // 512                # matmul chunks per score tile
    NQT = seq // QW               # q tiles
    EXP = mybir.ActivationFunctionType.Exp

    wpool = ctx.enter_context(tc.tile_pool(name="w", bufs=1))
    xpool = ctx.enter_context(tc.tile_pool(name="x", bufs=2))
    perm = ctx.enter_context(tc.tile_pool(name="perm", bufs=1))
    epool = ctx.enter_context(tc.tile_pool(name="e", bufs=12))
    small = ctx.enter_context(tc.tile_pool(name="small", bufs=2))
    opool = ctx.enter_context(tc.tile_pool(name="o", bufs=2))
    dpool = ctx.enter_context(tc.tile_pool(name="d", bufs=3, space="DRAM"))
    psS = ctx.enter_context(tc.tile_pool(name="psS", bufs=2, space="PSUM"))
    psC = ctx.enter_context(tc.tile_pool(name="psC", bufs=1, space="PSUM"))

    # --- persistent weights / biases ---
    wq_sb = wpool.tile([P, DC, G], BF16, name="wq_sb")
    wk_sb = wpool.tile([P, DC, G], BF16, name="wk_sb")
    wv_sb = wpool.tile([P, DC, G], BF16, name="wv_sb")
    wo_sb = wpool.tile([P, FC, D], BF16, name="wo_sb")
    bq_sb = wpool.tile([P, FC], F32, name="bq_sb")
    bk_sb = wpool.tile([P, FC], F32, name="bk_sb")
    bv_sb = wpool.tile([P, G], F32, name="bv_sb")

    # warm-up tile for the PE bridge over the initial DMA wait: memset only
    # one element (the DVE's first instruction) so the warm matmuls depend on
    # nothing else; the rest is garbage that lands in a never-read psum tile.
    warm_sb = wpool.tile([P, 512], BF16, name="warm_sb")
    nc.vector.memset(warm_sb[:, 0:1], 0.0)

    # --- persistent activations ---
    # QT/KT pack each head pair on the partition axis: even head of pair fc
    # on partitions 0..63, odd head on 64..127. Score matmuls slice the
    # 64-partition half directly (K=64 row tiles) so no zero padding is
    # needed anywhere in KT.
    QT = perm.tile([P, FC, seq], BF16, name="QT")
    KT = perm.tile([P, FC, seq], BF16, name="KT")
    # V: one 128-wide stripe per head: cols 0..63 = V_h, col 64 = ones
    # (softmax denominator rides the PV matmul), cols 65..127 = zeros
    # (pads M to 128 to keep the PE activity monitor at full clock).
    V = perm.tile([P, KC, NH * P], BF16, name="V")
    CT = perm.tile([P, FC, seq], BF16, name="CT")
    V4 = V.rearrange("p kc (h c) -> p kc h c", h=NH)
    nc.vector.memset(V4[:, :, :, DH:], 0.0)
    nc.vector.memset(V4[:, :, :, DH:DH + 1], 1.0)

    # --- input streaming ---
    def xfetch(tname, qc):
        t = xpool.tile([P, DC, 512], BF16, name=f"x{tname}{qc}", tag="xt")
        nc.sync.dma_start(t[:], io["x" + tname + "T"][qc].rearrange(
            "p (dc s) -> p dc s", s=512))
        return t

    # projections borrow the score-psum ring (the PE runs one thing at a
    # time, so a wedge simply takes one ring turn).
    def wedge_ps(name):
        return psS.tile([P, QW], F32, name=name, tag="score")

    def proj_kq(tname, qc, fcs, xt):
        wsb, bsb, dst = (wq_sb, bq_sb, QT) if tname == "q" else \
            (wk_sb, bk_sb, KT)
        for fc in fcs:
            ps = wedge_ps(f"p{tname}{qc}{fc}")
            for dc in range(DC):
                nc.tensor.matmul(
                    ps[:, 0:512], wsb[:, dc, fc * P:(fc + 1) * P], xt[:, dc, :],
                    start=(dc == 0), stop=(dc == DC - 1))
            nc.vector.tensor_scalar_add(
                out=dst[:, fc, qc * 512:(qc + 1) * 512], in0=ps[:, 0:512],
                scalar1=bsb[:, fc:fc + 1])

    def proj_v1(kc, xt):
        s4 = kc % 4
        ps = wedge_ps(f"pv{kc}")
        for dc in range(DC):
            nc.tensor.matmul(
                ps[:, 0:512], xt[:, dc, s4 * P:(s4 + 1) * P], wv_sb[:, dc, :],
                start=(dc == 0), stop=(dc == DC - 1))
        nc.vector.tensor_add(
            out=V[:, kc].rearrange("p (h c) -> p h c", h=NH)[:, :, 0:DH],
            in0=ps[:, 0:512].rearrange("p (h c) -> p h c", h=NH),
            in1=bv_sb.rearrange("p (h c) -> p h c", h=NH))

    # --- attention helpers ---
    def scores_pair(qt, hp, kc):
        """Row-tiled K=64 score matmuls for both heads of pair hp: the even
        head runs in array rows 0..63 (tile (0,0)), the odd head in rows
        64..127 (tile (64,0)), concurrently. Returns (esA, esB)."""
        spsA = psS.tile([P, QW], F32, name=f"sA{qt}p{hp}k{kc}", tag="score")
        spsB = psS.tile([P, QW], F32, name=f"sB{qt}p{hp}k{kc}", tag="score")
        for i in range(NI):
            q0 = qt * QW + i * 512
            nc.tensor.matmul(
                spsA[:, i * 512:(i + 1) * 512],
                KT[0:DH, hp, kc * P:(kc + 1) * P],
                QT[0:DH, hp, q0:q0 + 512],
                start=True, stop=True)
            nc.tensor.matmul(
                spsB[:, i * 512:(i + 1) * 512],
                KT[DH:P, hp, kc * P:(kc + 1) * P],
                QT[DH:P, hp, q0:q0 + 512],
                start=True, stop=True)
        esA = epool.tile([P, QW], BF16, name=f"eA{qt}p{hp}k{kc}", tag="es")
        nc.scalar.activation(esA[:], spsA[:], EXP, scale=SCALE)
        esB = epool.tile([P, QW], BF16, name=f"eB{qt}p{hp}k{kc}", tag="es")
        nc.scalar.activation(esB[:], spsB[:], EXP, scale=SCALE)
        return esA, esB

    def emit_pv(h, kc, cps, es):
        for i in range(NI):
            nc.tensor.matmul(
                cps[i][:, :],
                V[:, kc, h * P:(h + 1) * P],
                es[:, i * 512:(i + 1) * 512],
                start=(kc == 0), stop=(kc == KC - 1))

    def emit_normalize(qt, h, cps):
        p0 = (h % 2) * DH
        fcH = h // 2
        for i in range(NI):
            q0 = qt * QW + i * 512
            l1 = small.tile([P, 512], BF16, name=f"l{qt}h{h}i{i}", tag="l1")
            nc.vector.tensor_copy(out=l1[DH:DH + 1, :], in_=cps[i][DH:DH + 1, :])
            csb = small.tile([DH, 512], F32, name=f"cs{qt}h{h}i{i}", tag="csb")
            nc.vector.tensor_copy(out=csb[:], in_=cps[i][0:DH, :])
            rd = dpool.tile([1, 512], BF16, name=f"rd{qt}h{h}i{i}", tag="rd")
            nc.sync.dma_start(rd[:], l1[DH:DH + 1, :])
            lbb = small.tile([DH, 512], F32, name=f"lb{qt}h{h}i{i}", tag="lbb")
            nc.gpsimd.dma_start(lbb[:], rd[0].partition_broadcast(DH))
            rbb = lbb
            nc.vector.reciprocal_approx_fast(rbb[:], lbb[:])
            if p0 == 0:
                nc.vector.tensor_mul(out=CT[0:DH, fcH, q0:q0 + 512],
                                     in0=csb[:], in1=rbb[:])
            else:
                tmp = small.tile([P, 512], BF16, name=f"t{qt}h{h}i{i}", tag="tmp")
                nc.vector.tensor_mul(out=tmp[0:DH, :],
                                     in0=csb[:], in1=rbb[:])
                nc.sync.dma_start(CT[DH:2 * DH, fcH, q0:q0 + 512], tmp[0:DH, :])

    def outproj(qt, s8s=None):
        for s8 in (range(QW // P) if s8s is None else s8s):
            sc = qt * (QW // P) + s8
            ops = wedge_ps(f"op{sc}")
            for oc in range(D // 512):
                o0 = oc * 512
                for fc in range(FC):
                    nc.tensor.matmul(
                        ops[:, o0:o0 + 512], CT[:, fc, sc * P:(sc + 1) * P],
                        wo_sb[:, fc, o0:o0 + 512],
                        start=(fc == 0), stop=(fc == FC - 1))
            osb = opool.tile([P, D], F32, name=f"ob{sc}", tag="ob")
            nc.vector.tensor_copy(out=osb[:], in_=ops[:, 0:D])
            nc.sync.dma_start(io["out"][sc * P:(sc + 1) * P, :], osb[:])

    def attn_pair(qt, hp, wedges):
        """One head pair of one q tile: GK-batched score/PV phase blocks.
        `wedges` is a list of zero-arg emitters woven into the 128-mode
        blocks (one per block at most)."""
        hA, hB = 2 * hp, 2 * hp + 1
        cpsA = [psC.tile([P, 512], F32, name=f"cA{qt}p{hp}i{i}", tag=f"cA{i}")
                for i in range(NI)]
        cpsB = [psC.tile([P, 512], F32, name=f"cB{qt}p{hp}i{i}", tag=f"cB{i}")
                for i in range(NI)]
        for g in range(KC // GK):
            pend = []
            for kc in range(g * GK, (g + 1) * GK):
                pend.append((kc, scores_pair(qt, hp, kc)))
            for kc, (esA, esB) in pend:
                emit_pv(hA, kc, cpsA, esA)
                emit_pv(hB, kc, cpsB, esB)
            if wedges and g % ((KC // GK) // 2) == (KC // GK) // 2 - 1:
                wedges.pop(0)()
        # odd head first: its CT write goes through a DMA hop that then
        # drains under the even head's plain vector normalize
        emit_normalize(qt, hB, cpsB)
        emit_normalize(qt, hA, cpsA)

    # --- program order ---
    NQC = seq // 512
    # warm-up matmuls bridge the initial DMA wait so the PE activity monitor
    # never clock-gates the array; they depend only on a 1-element memset.
    wps = psS.tile([P, QW], F32, name="warm_ps", tag="score")
    for i in range(14):
        nc.tensor.matmul(wps[:, 0:512], warm_sb[:, 0:P], warm_sb[:],
                         start=True, stop=True)
    # K chunk 0 then Q (both window-0 chunks) come first so pair-0 scores
    # and their ScalarE exps can start ~30us in, overlapping the K tail and
    # the V projection weave instead of idling through them.
    nc.sync.dma_start(bk_sb[:], io["bkc"][:])
    nc.sync.dma_start(wk_sb[:], io["wkT"][:])
    xk_t = {0: xfetch("k", 0)}
    xq_t = {}
    xv_t = {}
    nc.sync.dma_start(bq_sb[:], io["bqc"][:])
    nc.sync.dma_start(wq_sb[:], io["wqT"][:])
    proj_kq("k", 0, range(FC), xk_t.pop(0))
    xq_t[0] = xfetch("q", 0)
    nc.sync.dma_start(bv_sb[:], io["bvc"][:])
    nc.sync.dma_start(wv_sb[:], io["wvT"][:])
    proj_kq("q", 0, range(FC), xq_t.pop(0))
    xq_t[1] = xfetch("q", 1)
    xk_t[1] = xfetch("k", 1)
    proj_kq("q", 1, range(FC), xq_t.pop(1))
    # pair-0 scores for the already-projected K range are woven into the K
    # tail so ScalarE starts its exp stream ~25us earlier. At most 6 kc are
    # pre-scored: the es ring (12) must not need freeing before PV exists.
    es_pre = {}
    for qc in range(1, NQC):
        for kc in (2 * (qc - 1), 2 * (qc - 1) + 1):
            es_pre[kc] = scores_pair(0, 0, kc)
        proj_kq("k", qc, range(FC), xk_t.pop(qc))
        if qc + 1 < NQC:
            xk_t[qc + 1] = xfetch("k", qc + 1)
        if qc == NQC - 1:
            xv_t[0] = xfetch("v", 0)
            xv_t[1] = xfetch("v", 1)

    # qt 0, pair 0: scores interleave with the V projection (V[kc] is ready
    # group-by-group); later pairs weave the Q-tail projections.
    hA, hB = 0, 1
    cpsA = [psC.tile([P, 512], F32, name=f"cA0p0i{i}", tag=f"cA{i}")
            for i in range(NI)]
    cpsB = [psC.tile([P, 512], F32, name=f"cB0p0i{i}", tag=f"cB{i}")
            for i in range(NI)]
    def pv_slice(s4):
        qc, s = divmod(s4, 4)
        if qc not in xv_t:
            xv_t[qc] = xfetch("v", qc)
        proj_v1(qc * 4 + s, xv_t[qc])
        if s == 3:
            xv_t.pop(qc)
            if qc + 2 < NQC and qc + 2 not in xv_t:
                xv_t[qc + 2] = xfetch("v", qc + 2)
    for g in range(KC // GK):
        pend = []
        for kc in range(g * GK, (g + 1) * GK):
            pend.append((kc, es_pre.pop(kc) if kc in es_pre
                         else scores_pair(0, 0, kc)))
        # two V-projection chunk slices per block keep V exactly ahead of
        # the PV consumer (kc chunk 2g+1 needs V chunks <= 2g+1)
        pv_slice(2 * g)
        pv_slice(2 * g + 1)
        for kc, (esA, esB) in pend:
            emit_pv(hA, kc, cpsA, esA)
            emit_pv(hB, kc, cpsB, esB)
        if g == 0:
            nc.sync.dma_start(wo_sb[:], io["woT"][:])
    emit_normalize(0, hB, cpsB)
    emit_normalize(0, hA, cpsA)

    # Q-tail projections (qc 2..NQC-1), split into half-projection wedges
    qwedges = []
    for qc in range(2, NQC):
        for fc2 in range(2):
            def qw(qc=qc, fc2=fc2):
                if fc2 == 0 and qc not in xq_t:
                    xq_t[qc] = xfetch("q", qc)
                proj_kq("q", qc, [2 * fc2, 2 * fc2 + 1], xq_t[qc])
                if fc2 == 1:
                    xq_t.pop(qc)
            qwedges.append(qw)
    for hp in range(1, NP):
        w = qwedges[2 * (hp - 1):2 * hp]
        attn_pair(0, hp, w)

    # remaining q tiles: previous tile's output projection woven into the
    # 128-mode PV blocks (one seq-chunk wedge per block)
    for qt in range(1, NQT):
        owedges = []
        for s8 in range(QW // P):
            def ow(qt=qt, s8=s8):
                outproj(qt - 1, [s8])
            owedges.append(ow)
        # pair order: last pair ends on its even head's normalize (plain
        # vector CT write, no DMA hop) to shorten the final outproj tail
        for hp in range(NP):
            w = owedges[2 * hp:2 * hp + 2]
            attn_pair(qt, hp, w)
    # warm filler bridges the last normalize-chain drain so the activity
    # monitor keeps the PE at full clock into the final output projection
    wps2 = psS.tile([P, QW], F32, name="warm_ps2", tag="score")
    for i in range(20):
        nc.tensor.matmul(wps2[:, 0:512], warm_sb[:, 0:P], warm_sb[:],
                         start=True, stop=True)
    outproj(NQT - 1)


def build_program(seq=S, num_devices=8):
    nc = bacc.Bacc("TRN2", target_bir_lowering=False, debug=False,
                   num_devices=num_devices)
    nqc = seq // 512
    io = {
        "xqT": nc.dram_tensor("xqT", (nqc, P, DC * 512), BF16, kind="ExternalInput").ap(),
        "xkT": nc.dram_tensor("xkT", (nqc, P, DC * 512), BF16, kind="ExternalInput").ap(),
        "xvT": nc.dram_tensor("xvT", (nqc, P, DC * 512), BF16, kind="ExternalInput").ap(),
        "wqT": nc.dram_tensor("wqT", (P, DC, G), BF16, kind="ExternalInput").ap(),
        "wkT": nc.dram_tensor("wkT", (P, DC, G), BF16, kind="ExternalInput").ap(),
        "wvT": nc.dram_tensor("wvT", (P, DC, G), BF16, kind="ExternalInput").ap(),
        "woT": nc.dram_tensor("woT", (P, FC, D), BF16, kind="ExternalInput").ap(),
        "bqc": nc.dram_tensor("bqc", (P, FC), F32, kind="ExternalInput").ap(),
        "bkc": nc.dram_tensor("bkc", (P, FC), F32, kind="ExternalInput").ap(),
        "bvc": nc.dram_tensor("bvc", (P, G), F32, kind="ExternalInput").ap(),
        "out": nc.dram_tensor("out", (seq, D), F32, kind="ExternalOutput").ap(),
    }
    with tile.TileContext(nc) as tc:
        with ExitStack() as ctx:
            _emit(ctx, tc, io, seq)
    nc.compile()
    return nc


_PROG = None


def _get_prog():
    global _PROG
    if _PROG is None:
        _PROG = build_program()
    return _PROG


def make_in_maps(q, k, v, wq, bq, wk, bk, wv, bv, wo):
    bf16 = ml_dtypes.bfloat16
    f32 = np.float32
    NQC = S // 512

    def xdev(x):
        t = x.T.reshape(DC, P, NQC, 512).transpose(2, 1, 0, 3)
        return np.ascontiguousarray(t).astype(bf16).reshape(NQC, P, DC * 512)

    def wdev(w):
        return np.ascontiguousarray(
            w.T.reshape(DC, P, G).transpose(1, 0, 2)).astype(bf16)

    def bdev(b):
        return np.ascontiguousarray(b.reshape(FC, P).T).astype(f32)

    xT = []
    for b in range(B):
        xT.append((xdev(q[b]), xdev(k[b]), xdev(v[b])))
    halves = []
    for hh in range(2):
        rows = slice(hh * G, (hh + 1) * G)
        halves.append({
            "wqT": wdev(wq[rows, :]),
            "wkT": wdev(wk[rows, :]),
            "wvT": wdev(wv[rows, :]),
            "woT": np.ascontiguousarray(
                wo[:, rows].T.reshape(FC, P, D).transpose(1, 0, 2)).astype(bf16),
            "bqc": bdev(np.asarray(bq[rows])),
            "bkc": bdev(np.asarray(bk[rows])),
            "bvc": np.ascontiguousarray(
                np.broadcast_to(np.asarray(bv[rows]), (P, G))).astype(f32),
        })
    in_maps = []
    for c in range(8):
        b, hh = c // 2, c % 2
        m = dict(halves[hh])
        m["xqT"], m["xkT"], m["xvT"] = xT[b]
        in_maps.append(m)
    return in_maps


def run_with_results(q, k, v, wq, bq, wk, bk, wv, bv, wo, bo, **kw):
    nc = _get_prog()
    in_maps = make_in_maps(np.asarray(q, np.float32), np.asarray(k, np.float32),
                           np.asarray(v, np.float32), np.asarray(wq, np.float32),
                           np.asarray(bq, np.float32), np.asarray(wk, np.float32),
                           np.asarray(bk, np.float32), np.asarray(wv, np.float32),
                           np.asarray(bv, np.float32), np.asarray(wo, np.float32))
    res = run_bass_kernel_spmd(nc, in_maps, core_ids=list(range(8)), **kw)
    parts = [res.results[c]["out"] for c in range(8)]
    bo = np.asarray(bo, np.float32)
    out = np.stack([parts[2 * b] + parts[2 * b + 1] + bo for b in range(B)])
    return out.astype(np.float32), res


def kernel(q, k, v, wq, bq, wk, bk, wv, bv, wo, bo):
    out, _ = run_with_results(q, k, v, wq, bq, wk, bk, wv, bv, wo, bo)
    return out


# revision 20
# speedup vs baseline: 1.0011x; 1.0011x over previous
"""Multi-head attention (B=4, S=2048, D=1024, H=16, dk=dv=64) on 8 TRN2 NeuronCores.

Sharding: batch x head-half. Core c handles batch b = c//2 and heads
hh*8..hh*8+8 where hh = c%2. Each core computes its 8 heads' attention plus
the partial output projection (row-parallel fc); the host sums the two
partials per batch and adds the output bias.

Device algorithm per core (matmul inputs bf16, PSUM fp32):
  - host pre-transposes inputs (xT) so every matmul contracts over partitions.
  - Q^T/K^T laid out [feat, seq] with HEAD PAIRS packed on the partition axis
    (even head of a pair on partitions 0..63, odd head on 64..127).
  - scores S^T = K^T' Q^T per head run as K=64 matmuls in PE ROW TILES
    (tile_position (0,0) / (64,0)): the two heads of a pair execute
    CONCURRENTLY in the two array halves, halving score matmul time.
  - scores are emitted in kc-groups of 4 per pair, alternating with the
    128-mode PV block for that group, to amortize PE tiling-mode switches.
  - V augmented with a ones column per head: the softmax denominator rides
    the PV matmul; exp on ScalarE reads score PSUM directly (no max-sub).
  - normalize via reciprocal + partition broadcast (DRAM bounce, off the
    critical path); out_partial = C^T.T @ Wo_c^T, fp32 to DRAM.

PSUM budget (8 banks): score/wedge ring 2x[128,1024] (4 banks; the
projection and output-projection wedges BORROW this ring since the PE can
only run one thing at a time) + PV accumulators 4x[128,512] (4 banks,
two heads of a pair in flight).
"""

import sys

if "/opt/trn_rl_repo" not in sys.path:
    sys.path.insert(0, "/opt/trn_rl_repo")

from contextlib import ExitStack

import ml_dtypes
import numpy as np

import concourse.bass as bass
import concourse.tile as tile
from concourse import bacc, mybir
from concourse.bass_utils import run_bass_kernel_spmd

BF16 = mybir.dt.bfloat16
F32 = mybir.dt.float32
P = 128

B, S, D = 4, 2048, 1024
H, DH = 16, 64
G = 512          # head-group width per core: 8 heads x 64
NH = G // DH     # 8 heads per core
NP = NH // 2     # head pairs per core
DC = D // P      # contraction chunks over model dim
FC = G // P      # feat chunks of the head-group width
SCALE = 1.0 / 8.0  # 1/sqrt(dk)
GK = 2           # kc chunks per scores/PV phase block (mode-switch batching)


def _emit(ctx, tc, io, seq):
    nc = tc.nc
    KC = seq // P                 # key chunks
    QW = min(1024, seq)           # q width per score-psum tile (2 PSUM banks)
    NI = QW // 512                # matmul chunks per score tile
    NQT = seq // QW               # q tiles
    EXP = mybir.ActivationFunctionType.Exp

    wpool = ctx.enter_context(tc.tile_pool(name="w", bufs=1))
    xpool = ctx.enter_context(tc.tile_pool(name="x", bufs=2))
    perm = ctx.enter_context(tc.tile_pool(name="perm", bufs=1))
    epool = ctx.enter_context(tc.tile_pool(name="e", bufs=16))
    small = ctx.enter_context(tc.tile_pool(name="small", bufs=2))
    opool = ctx.enter_context(tc.tile_pool(name="o", bufs=2))
    dpool = ctx.enter_context(tc.tile_pool(name="d", bufs=3, space="DRAM"))
    psS = ctx.enter_context(tc.tile_pool(name="psS", bufs=2, space="PSUM"))
    psC = ctx.enter_context(tc.tile_pool(name="psC", bufs=1, space="PSUM"))

    # --- persistent weights / biases ---
    wq_sb = wpool.tile([P, DC, G], BF16, name="wq_sb")
    wk_sb = wpool.tile([P, DC, G], BF16, name="wk_sb")
    wv_sb = wpool.tile([P, DC, G], BF16, name="wv_sb")
    wo_sb = wpool.tile([P, FC, D], BF16, name="wo_sb")
    bq_sb = wpool.tile([P, FC], F32, name="bq_sb")
    bk_sb = wpool.tile([P, FC], F32, name="bk_sb")
    bv_sb = wpool.tile([P, G], F32, name="bv_sb")

    # warm-up tile for the PE bridge over the initial DMA wait: memset only
    # one element (the DVE's first instruction) so the warm matmuls depend on
    # nothing else; the rest is garbage that lands in a never-read psum tile.
    warm_sb = wpool.tile([P, 512], BF16, name="warm_sb")
    nc.vector.memset(warm_sb[:, 0:1], 0.0)

    # --- persistent activations ---
    # QT/KT pack each head pair on the partition axis: even head of pair fc
    # on partitions 0..63, odd head on 64..127. Score matmuls slice the
    # 64-partition half directly (K=64 row tiles) so no zero padding is
    # needed anywhere in KT.
    QT = perm.tile([P, FC, seq], BF16, name="QT")
    KT = perm.tile([P, FC, seq], BF16, name="KT")
    # V: one 128-wide stripe per head: cols 0..63 = V_h, col 64 = ones
    # (softmax denominator rides the PV matmul), cols 65..127 = zeros
    # (pads M to 128 to keep the PE activity monitor at full clock).
    V = perm.tile([P, KC, NH * P], BF16, name="V")
    CT = perm.tile([P, FC, seq], BF16, name="CT")
    V4 = V.rearrange("p kc (h c) -> p kc h c", h=NH)
    nc.vector.memset(V4[:, :, :, DH:], 0.0)
    nc.vector.memset(V4[:, :, :, DH:DH + 1], 1.0)

    # --- input streaming ---
    def xfetch(tname, qc):
        t = xpool.tile([P, DC, 512], BF16, name=f"x{tname}{qc}", tag="xt")
        nc.sync.dma_start(t[:], io["x" + tname + "T"][qc].rearrange(
            "p (dc s) -> p dc s", s=512))
        return t

    # projections borrow the score-psum ring (the PE runs one thing at a
    # time, so a wedge simply takes one ring turn).
    def wedge_ps(name):
        return psS.tile([P, QW], F32, name=name, tag="score")

    def proj_kq(tname, qc, fcs, xt):
        wsb, bsb, dst = (wq_sb, bq_sb, QT) if tname == "q" else \
            (wk_sb, bk_sb, KT)
        for fc in fcs:
            ps = wedge_ps(f"p{tname}{qc}{fc}")
            for dc in range(DC):
                nc.tensor.matmul(
                    ps[:, 0:512], wsb[:, dc, fc * P:(fc + 1) * P], xt[:, dc, :],
                    start=(dc == 0), stop=(dc == DC - 1))
            nc.vector.tensor_scalar_add(
                out=dst[:, fc, qc * 512:(qc + 1) * 512], in0=ps[:, 0:512],
                scalar1=bsb[:, fc:fc + 1])

    def proj_v1(kc, xt):
        s4 = kc % 4
        ps = wedge_ps(f"pv{kc}")
        for dc in range(DC):
            nc.tensor.matmul(
                ps[:, 0:512], xt[:, dc, s4 * P:(s4 + 1) * P], wv_sb[:, dc, :],
                start=(dc == 0), stop=(dc == DC - 1))
        nc.vector.tensor_add(
            out=V[:, kc].rearrange("p (h c) -> p h c", h=NH)[:, :, 0:DH],
            in0=ps[:, 0:512].rearrange("p (h c) -> p h c", h=NH),
            in1=bv_sb.rearrange("p (h c) -> p h c", h=NH))

    # --- attention helpers ---
    def scores_pair(qt, hp, kc):
        """Row-tiled K=64 score matmuls for both heads of pair hp: the even
        head runs in array rows 0..63 (tile (0,0)), the odd head in rows
        64..127 (tile (64,0)), concurrently. Returns (esA, esB)."""
        spsA = psS.tile([P, QW], F32, name=f"sA{qt}p{hp}k{kc}", tag="score")
        spsB = psS.tile([P, QW], F32, name=f"sB{qt}p{hp}k{kc}", tag="score")
        for i in range(NI):
            q0 = qt * QW + i * 512
            nc.tensor.matmul(
                spsA[:, i * 512:(i + 1) * 512],
                KT[0:DH, hp, kc * P:(kc + 1) * P],
                QT[0:DH, hp, q0:q0 + 512],
                start=True, stop=True)
            nc.tensor.matmul(
                spsB[:, i * 512:(i + 1) * 512],
                KT[DH:P, hp, kc * P:(kc + 1) * P],
                QT[DH:P, hp, q0:q0 + 512],
                start=True, stop=True)
        esA = epool.tile([P, QW], BF16, name=f"eA{qt}p{hp}k{kc}", tag="es")
        nc.scalar.activation(esA[:], spsA[:], EXP, scale=SCALE)
        esB = epool.tile([P, QW], BF16, name=f"eB{qt}p{hp}k{kc}", tag="es")
        nc.scalar.activation(esB[:], spsB[:], EXP, scale=SCALE)
        return esA, esB

    def emit_pv(h, kc, cps, es):
        for i in range(NI):
            nc.tensor.matmul(
                cps[i][:, :],
                V[:, kc, h * P:(h + 1) * P],
                es[:, i * 512:(i + 1) * 512],
                start=(kc == 0), stop=(kc == KC - 1))

    def emit_normalize(qt, h, cps):
        p0 = (h % 2) * DH
        fcH = h // 2
        for i in range(NI):
            q0 = qt * QW + i * 512
            l1 = small.tile([P, 512], BF16, name=f"l{qt}h{h}i{i}", tag="l1")
            nc.vector.tensor_copy(out=l1[DH:DH + 1, :], in_=cps[i][DH:DH + 1, :])
            csb = small.tile([DH, 512], F32, name=f"cs{qt}h{h}i{i}", tag="csb")
            nc.vector.tensor_copy(out=csb[:], in_=cps[i][0:DH, :])
            rd = dpool.tile([1, 512], BF16, name=f"rd{qt}h{h}i{i}", tag="rd")
            nc.sync.dma_start(rd[:], l1[DH:DH + 1, :])
            lbb = small.tile([DH, 512], F32, name=f"lb{qt}h{h}i{i}", tag="lbb")
            nc.gpsimd.dma_start(lbb[:], rd[0].partition_broadcast(DH))
            rbb = lbb
            nc.vector.reciprocal_approx_fast(rbb[:], lbb[:])
            if p0 == 0:
                nc.vector.tensor_mul(out=CT[0:DH, fcH, q0:q0 + 512],
                                     in0=csb[:], in1=rbb[:])
            else:
                tmp = small.tile([P, 512], BF16, name=f"t{qt}h{h}i{i}", tag="tmp")
                nc.vector.tensor_mul(out=tmp[0:DH, :],
                                     in0=csb[:], in1=rbb[:])
                nc.sync.dma_start(CT[DH:2 * DH, fcH, q0:q0 + 512], tmp[0:DH, :])

    def outproj(qt, s8s=None):
        for s8 in (range(QW // P) if s8s is None else s8s):
            sc = qt * (QW // P) + s8
            ops = wedge_ps(f"op{sc}")
            for oc in range(D // 512):
                o0 = oc * 512
                for fc in range(FC):
                    nc.tensor.matmul(
                        ops[:, o0:o0 + 512], CT[:, fc, sc * P:(sc + 1) * P],
                        wo_sb[:, fc, o0:o0 + 512],
                        start=(fc == 0), stop=(fc == FC - 1))
            osb = opool.tile([P, D], F32, name=f"ob{sc}", tag="ob")
            nc.vector.tensor_copy(out=osb[:], in_=ops[:, 0:D])
            nc.sync.dma_start(io["out"][sc * P:(sc + 1) * P, :], osb[:])

    def attn_pair(qt, hp, wedges):
        """One head pair of one q tile: GK-batched score/PV phase blocks.
        `wedges` is a list of zero-arg emitters woven into the 128-mode
        blocks (one per block at most)."""
        hA, hB = 2 * hp, 2 * hp + 1
        cpsA = [psC.tile([P, 512], F32, name=f"cA{qt}p{hp}i{i}", tag=f"cA{i}")
                for i in range(NI)]
        cpsB = [psC.tile([P, 512], F32, name=f"cB{qt}p{hp}i{i}", tag=f"cB{i}")
                for i in range(NI)]
        for g in range(KC // GK):
            pend = []
            for kc in range(g * GK, (g + 1) * GK):
                pend.append((kc, scores_pair(qt, hp, kc)))
            for kc, (esA, esB) in pend:
                emit_pv(hA, kc, cpsA, esA)
                emit_pv(hB, kc, cpsB, esB)
            if wedges and g % ((KC // GK) // 2) == (KC // GK) // 2 - 1:
                wedges.pop(0)()
        # odd head first: its CT write goes through a DMA hop that then
        # drains under the even head's plain vector normalize
        emit_normalize(qt, hB, cpsB)
        emit_normalize(qt, hA, cpsA)

    # --- program order ---
    NQC = seq // 512
    # warm-up matmuls bridge the initial DMA wait so the PE activity monitor
    # never clock-gates the array; they depend only on a 1-element memset.
    wps = psS.tile([P, QW], F32, name="warm_ps", tag="score")
    for i in range(14):
        nc.tensor.matmul(wps[:, 0:512], warm_sb[:, 0:P], warm_sb[:],
                         start=True, stop=True)
    # K chunk 0 then Q (both window-0 chunks) come first so pair-0 scores
    # and their ScalarE exps can start ~30us in, overlapping the K tail and
    # the V projection weave instead of idling through them.
    nc.sync.dma_start(bk_sb[:], io["bkc"][:])
    nc.sync.dma_start(wk_sb[:], io["wkT"][:])
    xk_t = {0: xfetch("k", 0)}
    xq_t = {}
    xv_t = {}
    nc.sync.dma_start(bq_sb[:], io["bqc"][:])
    nc.sync.dma_start(wq_sb[:], io["wqT"][:])
    proj_kq("k", 0, range(FC), xk_t.pop(0))
    xq_t[0] = xfetch("q", 0)
    nc.sync.dma_start(bv_sb[:], io["bvc"][:])
    nc.sync.dma_start(wv_sb[:], io["wvT"][:])
    proj_kq("q", 0, range(FC), xq_t.pop(0))
    xq_t[1] = xfetch("q", 1)
    xk_t[1] = xfetch("k", 1)
    proj_kq("q", 1, range(FC), xq_t.pop(1))
    # pair-0 scores for the already-projected K range are woven into the K
    # tail so ScalarE starts its exp stream ~25us earlier. At most 6 kc are
    # pre-scored: the es ring (12) must not need freeing before PV exists.
    es_pre = {}
    for qc in range(1, NQC):
        for kc in (2 * (qc - 1), 2 * (qc - 1) + 1):
            es_pre[kc] = scores_pair(0, 0, kc)
        proj_kq("k", qc, range(FC), xk_t.pop(qc))
        if qc + 1 < NQC:
            xk_t[qc + 1] = xfetch("k", qc + 1)
        if qc == NQC - 1:
            xv_t[0] = xfetch("v", 0)
            xv_t[1] = xfetch("v", 1)

    # qt 0, pair 0: scores interleave with the V projection (V[kc] is ready
    # group-by-group); later pairs weave the Q-tail projections.
    hA, hB = 0, 1
    cpsA = [psC.tile([P, 512], F32, name=f"cA0p0i{i}", tag=f"cA{i}")
            for i in range(NI)]
    cpsB = [psC.tile([P, 512], F32, name=f"cB0p0i{i}", tag=f"cB{i}")
            for i in range(NI)]
    def pv_slice(s4):
        qc, s = divmod(s4, 4)
        if qc not in xv_t:
            xv_t[qc] = xfetch("v", qc)
        proj_v1(qc * 4 + s, xv_t[qc])
        if s == 3:
            xv_t.pop(qc)
            if qc + 2 < NQC and qc + 2 not in xv_t:
                xv_t[qc + 2] = xfetch("v", qc + 2)
    for g in range(KC // GK):
        pend = []
        for kc in range(g * GK, (g + 1) * GK):
            pend.append((kc, es_pre.pop(kc) if kc in es_pre
                         else scores_pair(0, 0, kc)))
        # two V-projection chunk slices per block keep V exactly ahead of
        # the PV consumer (kc chunk 2g+1 needs V chunks <= 2g+1)
        pv_slice(2 * g)
        pv_slice(2 * g + 1)
        for kc, (esA, esB) in pend:
            emit_pv(hA, kc, cpsA, esA)
            emit_pv(hB, kc, cpsB, esB)
        if g == 0:
            nc.sync.dma_start(wo_sb[:], io["woT"][:])
    emit_normalize(0, hB, cpsB)
    emit_normalize(0, hA, cpsA)

    # Q-tail projections (qc 2..NQC-1), split into half-projection wedges
    qwedges = []
    for qc in range(2, NQC):
        for fc2 in range(2):
            def qw(qc=qc, fc2=fc2):
                if fc2 == 0 and qc not in xq_t:
                    xq_t[qc] = xfetch("q", qc)
                proj_kq("q", qc, [2 * fc2, 2 * fc2 + 1], xq_t[qc])
                if fc2 == 1:
                    xq_t.pop(qc)
            qwedges.append(qw)
    for hp in range(1, NP):
        w = qwedges[2 * (hp - 1):2 * hp]
        attn_pair(0, hp, w)

    # remaining q tiles: previous tile's output projection woven into the
    # 128-mode PV blocks (one seq-chunk wedge per block)
    for qt in range(1, NQT):
        owedges = []
        for s8 in range(QW // P):
            def ow(qt=qt, s8=s8):
                outproj(qt - 1, [s8])
            owedges.append(ow)
        # pair order: last pair ends on its even head's normalize (plain
        # vector CT write, no DMA hop) to shorten the final outproj tail
        for hp in range(NP):
            w = owedges[2 * hp:2 * hp + 2]
            attn_pair(qt, hp, w)
    # warm filler bridges the last normalize-chain drain so the activity
    # monitor keeps the PE at full clock into the final output projection
    wps2 = psS.tile([P, QW], F32, name="warm_ps2", tag="score")
    for i in range(20):
        nc.tensor.matmul(wps2[:, 0:512], warm_sb[:, 0:P], warm_sb[:],
                         start=True, stop=True)
    outproj(NQT - 1)


def build_program(seq=S, num_devices=8):
    nc = bacc.Bacc("TRN2", target_bir_lowering=False, debug=False,
                   num_devices=num_devices)
    nqc = seq // 512
    io = {
        "xqT": nc.dram_tensor("xqT", (nqc, P, DC * 512), BF16, kind="ExternalInput").ap(),
        "xkT": nc.dram_tensor("xkT", (nqc, P, DC * 512), BF16, kind="ExternalInput").ap(),
        "xvT": nc.dram_tensor("xvT", (nqc, P, DC * 512), BF16, kind="ExternalInput").ap(),
        "wqT": nc.dram_tensor("wqT", (P, DC, G), BF16, kind="ExternalInput").ap(),
        "wkT": nc.dram_tensor("wkT", (P, DC, G), BF16, kind="ExternalInput").ap(),
        "wvT": nc.dram_tensor("wvT", (P, DC, G), BF16, kind="ExternalInput").ap(),
        "woT": nc.dram_tensor("woT", (P, FC, D), BF16, kind="ExternalInput").ap(),
        "bqc": nc.dram_tensor("bqc", (P, FC), F32, kind="ExternalInput").ap(),
        "bkc": nc.dram_tensor("bkc", (P, FC), F32, kind="ExternalInput").ap(),
        "bvc": nc.dram_tensor("bvc", (P, G), F32, kind="ExternalInput").ap(),
        "out": nc.dram_tensor("out", (seq, D), F32, kind="ExternalOutput").ap(),
    }
    with tile.TileContext(nc) as tc:
        with ExitStack() as ctx:
            _emit(ctx, tc, io, seq)
    nc.compile()
    return nc


_PROG = None


def _get_prog():
    global _PROG
    if _PROG is None:
        _PROG = build_program()
    return _PROG


def make_in_maps(q, k, v, wq, bq, wk, bk, wv, bv, wo):
    bf16 = ml_dtypes.bfloat16
    f32 = np.float32
    NQC = S // 512

    def xdev(x):
        t = x.T.reshape(DC, P, NQC, 512).transpose(2, 1, 0, 3)
        return np.ascontiguousarray(t).astype(bf16).reshape(NQC, P, DC * 512)

    def wdev(w):
        return np.ascontiguousarray(
            w.T.reshape(DC, P, G).transpose(1, 0, 2)).astype(bf16)

    def bdev(b):
        return np.ascontiguousarray(b.reshape(FC, P).T).astype(f32)

    xT = []
    for b in range(B):
        xT.append((xdev(q[b]), xdev(k[b]), xdev(v[b])))
    halves = []
    for hh in range(2):
        rows = slice(hh * G, (hh + 1) * G)
        halves.append({
            "wqT": wdev(wq[rows, :]),
            "wkT": wdev(wk[rows, :]),
            "wvT": wdev(wv[rows, :]),
            "woT": np.ascontiguousarray(
                wo[:, rows].T.reshape(FC, P, D).transpose(1, 0, 2)).astype(bf16),
            "bqc": bdev(np.asarray(bq[rows])),
            "bkc": bdev(np.asarray(bk[rows])),
            "bvc": np.ascontiguousarray(
                np.broadcast_to(np.asarray(bv[rows]), (P, G))).astype(f32),
        })
    in_maps = []
    for c in range(8):
        b, hh = c // 2, c % 2
        m = dict(halves[hh])
        m["xqT"], m["xkT"], m["xvT"] = xT[b]
        in_maps.append(m)
    return in_maps


def run_with_results(q, k, v, wq, bq, wk, bk, wv, bv, wo, bo, **kw):
    nc = _get_prog()
    in_maps = make_in_maps(np.asarray(q, np.float32), np.asarray(k, np.float32),
                           np.asarray(v, np.float32), np.asarray(wq, np.float32),
                           np.asarray(bq, np.float32), np.asarray(wk, np.float32),
                           np.asarray(bk, np.float32), np.asarray(wv, np.float32),
                           np.asarray(bv, np.float32), np.asarray(wo, np.float32))
    res = run_bass_kernel_spmd(nc, in_maps, core_ids=list(range(8)), **kw)
    parts = [res.results[c]["out"] for c in range(8)]
    bo = np.asarray(bo, np.float32)
    out = np.stack([parts[2 * b] + parts[2 * b + 1] + bo for b in range(B)])
    return out.astype(np.float32), res


def kernel(q, k, v, wq, bq, wk, bk, wv, bv, wo, bo):
    out, _ = run_with_results(q, k, v, wq, bq, wk, bk, wv, bv, wo, bo)
    return out
